# revision 1
# baseline (speedup 1.0000x reference)
"""Trainium2 Bass kernel for residual-VQ autoencoder (nn_Autoencoder_45148696216751).

v2: fp8(e4m3) DoubleRow score sweeps + top-8 exact-rescore rescue.

Pipeline per core (data-parallel over tokens, 8 cores x 2048 tokens):
  encoder z_td[t,d] = x @ enc_w.T + b   (fp32 matmuls; exact, drives rescore)
  rT8[d,t] = fp8(transpose(z_td))       (PE transpose + cast; sweep lhsT)
  2x VQ stage:
     scores[t,k] = lam*(r.c - |c|^2/2) via fp8 DoubleRow matmuls
                   (codebook streamed as 6 fp8 rows: 4 data k-tiles + 2
                    bias-partial rows contracted with an all-ones pair)
     per-superchunk top-8 (max8 + max_index) -> m8buf/idxbuf
     global top-8 via max8+max_index over m8buf; absolute code ids via
     pool gather of idxf; ONE batched indirect-DMA gather of the 8
     candidate rows from a bias-padded table cbx[K, 520]
     exact rescore sc8[k] = q_k . z + bias_k (4x DVE tensor_tensor_reduce
     + 4x Pool scalar_tensor_tensor); winner row picked from SBUF via
     pool indirect_copy; PE transpose -> q1T accumulation, z_td update,
     rT8 regenerated from z_td for stage 2
  decoder out = q_sum @ dec_w.T + dec_b (f32r matmuls)
"""
import os, sys, types

os.environ.setdefault("NEURON_RT_RESET_CORES", "1")
sys.path.insert(0, '/opt/trn_rl_repo')
import numpy as np

import concourse.bass as bass
import concourse.tile as tile
from concourse import bacc, mybir
from concourse.bass_utils import run_bass_kernel_spmd
from concourse.masks import make_identity

f32 = mybir.dt.float32
f32r = mybir.dt.float32r
fp8 = mybir.dt.float8e4
i32 = mybir.dt.int32
u16 = mybir.dt.uint16
ALU = mybir.AluOpType
DR = mybir.MatmulPerfMode.DoubleRow

NCORES = 8
B, N, D = 4, 4096, 512
T = B * N                 # 16384 tokens
TL = T // NCORES          # 2048 tokens per core
K = 16384                 # codebook size
NT = TL // 128            # 16 token tiles per core
NJ = D // 128             # 4 contraction tiles
SC = 1024                 # superchunk (2 psum banks)
NSC = K // SC             # 16 superchunks
NR8 = 6                   # fp8 stream rows (4 cb + 2 bias-partial)
NUM_Q = 2
NCAND = 8                 # rescued candidates per token
NDVE = 4                  # tiles per group scanned by DVE (rest Pool)
PADW = 520                # cbx row: 512 cb + 1 bias + pad
LAM = 0.5                 # score scale so |lam*bias| fits fp8 range


def _ensure_axon_hook():
    """Register the NTFF profile hook (missing antenv.axon_hooks shim)."""
    if "antenv.axon_hooks" in sys.modules:
        return
    mod = types.ModuleType("antenv.axon_hooks")
    _h = [None]
    mod.set_axon_ntff_profile_hook = lambda h: _h.__setitem__(0, h)
    mod.get_axon_ntff_profile_hook = lambda: _h[0]
    sys.modules["antenv.axon_hooks"] = mod
    try:
        import antenv
        antenv.axon_hooks = mod
        from trn_agent_boot.trn_boot import _ntff_profile_via_ctypes
        hook = _ntff_profile_via_ctypes('/opt/axon/libaxon_pjrt.so')
        if hook is not None:
            mod.set_axon_ntff_profile_hook(hook)
    except Exception:
        pass


def _build():
    nc = bacc.Bacc("TRN2", target_bir_lowering=False, debug=False,
                   num_devices=NCORES)

    xT_d = nc.dram_tensor("xT", [128, NJ, TL], f32, kind="ExternalInput")
    cbs_d = nc.dram_tensor("cbs", [128, NR8, K], fp8, kind="ExternalInput")
    cbx_d = nc.dram_tensor("cbx", [K, PADW], f32, kind="ExternalInput")
    ewT_d = nc.dram_tensor("ewT", [128, NJ, D], f32, kind="ExternalInput")
    dwT_d = nc.dram_tensor("dwT", [128, NJ, D], f32r, kind="ExternalInput")
    ebf_d = nc.dram_tensor("ebf", [128, D], f32, kind="ExternalInput")
    db_d = nc.dram_tensor("db", [128, D], f32, kind="ExternalInput")
    ones8_d = nc.dram_tensor("ones8", [128, 2, 128], fp8, kind="ExternalInput")
    out_d = nc.dram_tensor("out", [TL, D], f32, kind="ExternalOutput")

    from contextlib import ExitStack
    with tile.TileContext(nc) as tc, ExitStack() as ctx:
        big = ctx.enter_context(tc.tile_pool(name="big", bufs=1))
        scrp = ctx.enter_context(tc.tile_pool(name="scr", bufs=2))
        pscr = ctx.enter_context(tc.tile_pool(name="pscr", bufs=2))
        smallp = ctx.enter_context(tc.tile_pool(name="small", bufs=8))
        q8p = ctx.enter_context(tc.tile_pool(name="q8p", bufs=2))
        qtp = ctx.enter_context(tc.tile_pool(name="qtp", bufs=3))
        outp = ctx.enter_context(tc.tile_pool(name="outp", bufs=2))
        psc = ctx.enter_context(tc.tile_pool(name="psc", bufs=3, space="PSUM"))
        psm = ctx.enter_context(tc.tile_pool(name="psm", bufs=2, space="PSUM"))

        # ---- persistent tiles
        rT8 = big.tile([128, NJ, TL], fp8)     # fp8 residual (transposed)
        q1T = big.tile([128, NJ, TL], f32r)    # q1T, later q_sumT
        z_td = big.tile([128, NT, D], f32)     # exact residual, [t, d] layout
        ewT = big.tile([128, NJ, D], f32)
        dwT = big.tile([128, NJ, D], f32r)
        ebf = big.tile([128, D], f32)
        db = big.tile([128, D], f32)
        ones8 = big.tile([128, 2, 128], fp8)
        ident = big.tile([128, 128], f32)
        m8buf = big.tile([128, NT, NSC * 8], f32)
        idxbuf = big.tile([128, NT, NSC * 8], u16)
        iota_off8 = big.tile([128, NSC * 8], f32)

        nc.sync.dma_start(ewT[:], ewT_d.ap())
        nc.sync.dma_start(dwT[:], dwT_d.ap())
        nc.sync.dma_start(ebf[:], ebf_d.ap())
        nc.sync.dma_start(db[:], db_d.ap())
        nc.sync.dma_start(ones8[:], ones8_d.ap())
        make_identity(nc, ident[:])
        nc.gpsimd.iota(iota_off8[:], pattern=[[SC, NSC], [0, 8]], base=0,
                       channel_multiplier=0,
                       allow_small_or_imprecise_dtypes=True)

        def emit_rt8(t):
            """rT8[:, :, t*128:(t+1)*128] = fp8(transpose(z_td[:, t, :]))."""
            tp = psm.tile([128, NJ, 128], f32, tag="m")
            for j in range(NJ):
                nc.tensor.transpose(tp[:, j, :],
                                    z_td[:, t, j * 128:(j + 1) * 128],
                                    ident[:])
            nc.vector.tensor_copy(rT8[:, :, t * 128:(t + 1) * 128], tp[:])

        # ---- encoder: z_td (exact fp32), rT8 via transpose
        HL = TL // 2
        with tc.tile_pool(name="xp", bufs=1) as xp:
            for half in range(2):
                xth = xp.tile([128, NJ, HL], f32, tag="x")
                nc.sync.dma_start(xth[:],
                                  xT_d.ap()[:, :, half * HL:(half + 1) * HL])
                for t2 in range(HL // 128):
                    t = half * (HL // 128) + t2
                    psz = psm.tile([128, 512], f32, tag="m")
                    for j in range(NJ):
                        nc.tensor.matmul(
                            psz[:], lhsT=xth[:, j, t2 * 128:(t2 + 1) * 128],
                            rhs=ewT[:, j, :],
                            start=(j == 0), stop=(j == NJ - 1))
                    nc.vector.tensor_add(z_td[:, t, :], psz[:], ebf[:])
                    emit_rt8(t)

        with tc.tile_pool(name="cbp", bufs=4) as cbp:

            scan_q = []

            def flush_scan():
                while scan_q:
                    t_, sc_, ps_ = scan_q.pop(0)
                    nc.vector.max_index(
                        out=idxbuf[:, t_, sc_ * 8:sc_ * 8 + 8],
                        in_max=m8buf[:, t_, sc_ * 8:sc_ * 8 + 8],
                        in_values=ps_[:])

            def emit_sweep_sc(tset, sc):
                cbt = cbp.tile([128, NR8, SC], fp8, tag="cbt")
                nc.sync.dma_start(cbt[:], cbs_d.ap()[:, :, sc * SC:(sc + 1) * SC])
                for t in tset:
                    tsl = slice(t * 128, (t + 1) * 128)
                    ps = psc.tile([128, SC], f32, tag="sc")
                    for h in range(SC // 512):
                        pslice = ps[:, h * 512:(h + 1) * 512]
                        cslice = slice(h * 512, (h + 1) * 512)
                        nc.tensor.matmul(
                            pslice, lhsT=rT8[:, 0:2, tsl],
                            rhs=cbt[:, 0:2, cslice],
                            start=True, stop=False, perf_mode=DR)
                        nc.tensor.matmul(
                            pslice, lhsT=rT8[:, 2:4, tsl],
                            rhs=cbt[:, 2:4, cslice],
                            start=False, stop=False, perf_mode=DR)
                        nc.tensor.matmul(
                            pslice, lhsT=ones8[:],
                            rhs=cbt[:, 4:6, cslice],
                            start=False, stop=True, perf_mode=DR)
                    m8s = m8buf[:, t, sc * 8:sc * 8 + 8]
                    nc.vector.max(out=m8s, in_=ps[:])
                    nc.vector.max_index(out=idxbuf[:, t, sc * 8:sc * 8 + 8],
                                        in_max=m8s, in_values=ps[:])

            comb = {}

            def emit_a(s, t):
                st = {}
                idxf = smallp.tile([128, NSC * 8], f32, tag="idxf")
                nc.gpsimd.tensor_copy(idxf[:], idxbuf[:, t, :])
                nc.gpsimd.tensor_add(idxf[:], idxf[:], iota_off8[:])
                g8 = smallp.tile([128, 8], f32, tag="g8")
                nc.vector.max(out=g8[:], in_=m8buf[:, t, :])
                cs = smallp.tile([128, NCAND], f32, tag="cs")
                junkm = scrp.tile([128, NSC * 8], f32, tag="jm")
                for k in range(NCAND):
                    nc.vector.scalar_tensor_tensor(
                        out=junkm[:], in0=m8buf[:, t, :],
                        scalar=g8[:, k:k + 1], in1=idxf[:],
                        op0=ALU.is_ge, op1=ALU.mult,
                        accum_out=cs[:, k:k + 1])
                candf = smallp.tile([128, NCAND], f32, tag="cf")
                nc.vector.tensor_copy(candf[:, 0:1], cs[:, 0:1])
                nc.vector.tensor_sub(candf[:, 1:NCAND], cs[:, 1:NCAND],
                                     cs[:, 0:NCAND - 1])
                cand32 = smallp.tile([128, NCAND], i32, tag="c32")
                nc.vector.tensor_copy(cand32[:], candf[:])
                q8 = q8p.tile([128, NCAND, PADW], f32, tag="q8")
                for k in range(NCAND):
                    nc.gpsimd.indirect_dma_start(
                        out=q8[:, k, :], out_offset=None, in_=cbx_d.ap(),
                        in_offset=bass.IndirectOffsetOnAxis(
                            ap=cand32[:, k:k + 1], axis=0))
                st["candf"], st["q8"] = candf, q8
                comb[t] = st

            def emit_b(s, t):
                st = comb[t]
                candf, q8 = st["candf"], st["q8"]
                sc8 = smallp.tile([128, NCAND], f32, tag="sc8")
                junk = scrp.tile([128, D], f32, tag="j")
                for k in range(4):
                    nc.vector.scalar_tensor_tensor(
                        out=junk[:], in0=q8[:, k, 0:D], scalar=1.0,
                        in1=z_td[:, t, :], op0=ALU.bypass, op1=ALU.mult,
                        accum_out=sc8[:, k:k + 1])
                for k in range(4, NCAND):
                    junk2 = pscr.tile([128, D], f32, tag="j2")
                    nc.gpsimd.tensor_mul(junk2[:], q8[:, k, 0:D],
                                         z_td[:, t, :])
                    nc.scalar.activation(
                        out=junk2[:], in_=junk2[:],
                        func=mybir.ActivationFunctionType.Copy,
                        accum_out=sc8[:, k:k + 1])
                nc.vector.tensor_add(sc8[:], sc8[:],
                                     q8[:, :, D:D + 1].squeeze())
                gm = smallp.tile([128, 1], f32, tag="gm")
                nc.vector.tensor_reduce(gm[:], sc8[:],
                                        axis=mybir.AxisListType.X,
                                        op=ALU.max)
                junk8 = smallp.tile([128, NCAND], f32, tag="j8")
                wf = smallp.tile([128, 1], f32, tag="wf")
                nc.vector.scalar_tensor_tensor(
                    out=junk8[:], in0=sc8[:], scalar=gm[:],
                    in1=candf[:], op0=ALU.is_ge, op1=ALU.mult,
                    accum_out=wf[:])
                wi32 = smallp.tile([128, 1], i32, tag="wi")
                nc.vector.tensor_copy(wi32[:], wf[:])
                qrow = qtp.tile([128, D], f32, tag="qt")
                nc.gpsimd.indirect_dma_start(
                    out=qrow[:], out_offset=None, in_=cbx_d.ap(),
                    in_offset=bass.IndirectOffsetOnAxis(ap=wi32[:], axis=0))
                st["qrow"] = qrow

            def emit_c(s, t):
                st = comb.pop(t)
                qrow = st["qrow"]
                tsl = slice(t * 128, (t + 1) * 128)
                tp4 = psm.tile([128, NJ, 128], f32, tag="m")
                for j in range(NJ):
                    nc.tensor.transpose(tp4[:, j, :],
                                        qrow[:, j * 128:(j + 1) * 128],
                                        ident[:])
                if s == 0:
                    nc.scalar.copy(q1T[:, :, tsl], tp4[:])
                    nc.vector.tensor_sub(z_td[:, t, :], z_td[:, t, :],
                                         qrow[:])
                    emit_rt8(t)
                else:
                    nc.vector.tensor_add(q1T[:, :, tsl],
                                         q1T[:, :, tsl], tp4[:])

            def pump(ps, tiles, r):
                n = len(tiles)
                if 0 <= r < n:
                    emit_a(ps, tiles[r])
                if 0 <= r - 2 < n:
                    emit_b(ps, tiles[r - 2])
                if 0 <= r - 5 < n:
                    emit_c(ps, tiles[r - 5])

            GROUPS = [range(0, 8), range(8, 16)]
            prev = None
            for s in range(NUM_Q):
                for tset in GROUPS:
                    for sc in range(NSC):
                        emit_sweep_sc(tset, sc)
                        if prev is not None:
                            pump(prev[0], prev[1], sc)
                    flush_scan()
                    if prev is not None:
                        for r in range(NSC, len(prev[1]) + 6):
                            pump(prev[0], prev[1], r)
                    prev = (s, list(tset))

        # ---- decoder: out[t, d'] = q_sumT.T @ dec_w.T + dec_b (f32r)
        for t in range(NT):
            pump(prev[0], prev[1], t)
            pso = psm.tile([128, 512], f32, tag="m")
            for j in range(NJ):
                nc.tensor.matmul(pso[:], lhsT=q1T[:, j, t * 128:(t + 1) * 128],
                                 rhs=dwT[:, j, :],
                                 start=(j == 0), stop=(j == NJ - 1))
            o_t = outp.tile([128, D], f32, tag="o")
            nc.vector.tensor_add(o_t[:], pso[:], db[:])
            nc.sync.dma_start(out_d.ap()[t * 128:(t + 1) * 128, :], o_t[:])

    nc.compile()
    return nc


_CACHE = {}


def _get_nc():
    if "nc" not in _CACHE:
        _ensure_axon_hook()
        _CACHE["nc"] = _build()
    return _CACHE["nc"]


def _host_prep(x, enc_w, enc_b, codebook, dec_w, dec_b):
    import ml_dtypes
    f8 = ml_dtypes.float8_e4m3

    x = np.asarray(x, np.float32)
    enc_w = np.asarray(enc_w, np.float32)
    enc_b = np.asarray(enc_b, np.float32)
    cb = np.ascontiguousarray(np.asarray(codebook, np.float32))
    dec_w = np.asarray(dec_w, np.float32)
    dec_b = np.asarray(dec_b, np.float32)

    flat = x.reshape(T, D)
    csq = (cb.astype(np.float64) ** 2).sum(-1).astype(np.float32)
    bias = (-0.5 * csq).astype(np.float32)

    # fp8 stream: rows 0-3 = lam*cb k-tiles, rows 4-5 = bias partials
    cbT = np.ascontiguousarray(cb.T)                      # [D, K]
    cbs = np.zeros((128, NR8, K), np.float32)
    cbs[:, :NJ, :] = (LAM * cbT).reshape(NJ, 128, K).transpose(1, 0, 2)
    rem = (LAM * bias).astype(np.float32).copy()
    parts = []
    for _ in range(4):
        p = np.asarray(rem, f8).astype(np.float32)
        parts.append(p)
        rem = rem - p
    cbs[0, NJ, :] = parts[0]
    cbs[1, NJ, :] = parts[1]
    cbs[0, NJ + 1, :] = parts[2]
    cbs[1, NJ + 1, :] = parts[3]
    cbs8 = np.asarray(cbs, f8)

    # padded gather/rescore table: [cb row | -csq/2 | pad]
    cbx = np.zeros((K, PADW), np.float32)
    cbx[:, :D] = cb
    cbx[:, D] = bias

    ewT = np.ascontiguousarray(
        enc_w.T.reshape(NJ, 128, D).transpose(1, 0, 2))   # [128, NJ, D]
    dwT = np.ascontiguousarray(
        dec_w.T.reshape(NJ, 128, D).transpose(1, 0, 2))
    ebf = np.ascontiguousarray(np.broadcast_to(enc_b, (128, D)))
    dbf = np.ascontiguousarray(np.broadcast_to(dec_b, (128, D)))
    ones8 = np.ones((128, 2, 128), f8)

    common = {"cbs": cbs8, "cbx": cbx, "ewT": ewT, "dwT": dwT,
              "ebf": ebf, "db": dbf, "ones8": ones8}

    in_maps = []
    for s in range(NCORES):
        shard = flat[s * TL:(s + 1) * TL]                 # [TL, D]
        xT = np.ascontiguousarray(
            shard.T.reshape(NJ, 128, TL).transpose(1, 0, 2))
        in_maps.append({"xT": xT, **common})
    return in_maps


def _run(inputs, trace=False):
    nc = _get_nc()
    in_maps = _host_prep(**inputs)
    res = run_bass_kernel_spmd(nc, in_maps, list(range(NCORES)), trace=trace)
    outs = [res.results[s]["out"] for s in range(NCORES)]
    full = np.concatenate(outs, axis=0).reshape(B, N, D)
    return full, res


def kernel(**inputs) -> np.ndarray:
    out, _ = _run(inputs, trace=False)
    return out


def kernel_traced(**inputs):
    out, res = _run(inputs, trace=True)
    return out, res



# revision 15
# speedup vs baseline: 1.0411x; 1.0411x over previous
"""Trainium2 Bass kernel for residual-VQ autoencoder (nn_Autoencoder_45148696216751).

v3: encoder/decoder folding + DVE-minimal rescue.

Per core (data-parallel over tokens, 8 cores x 2048 tokens):
  stage-1 sweep runs directly on fp8(x) against cbE = codebook @ enc_w
  (host-folded), so the exact f32 encoder (z = x @ enc_w.T + b) overlaps
  under the sweep instead of gating it.
  2x VQ stage:
     scores[t,k] = lam*(r.c - |c|^2/2) via fp8 DoubleRow matmuls
     (4 data + 2 bias-partial DR matmuls per 1024-code superchunk)
     DVE: max8 -> m8buf (top-8 per superchunk, exact f32) and
          find_index8 -> idxbuf (within-superchunk positions)
     rescue per token tile: eps-perturbed m8 -> global max8 + find_index8
     gives the top-8 SLOTS; gpsimd gathers absolute code ids by slot;
     ONE batched indirect-DMA gather of 6 candidate rows [cb|bias];
     exact f32 rescore dots on gpsimd/scalar; winner picked by is_ge
     match; winner row gathered from cbx (residual update) and from
     cbD = codebook @ dec_w.T (host-folded decoder) for the output
     accumulation out[t] = cbD[w1] + cbD[w2] + dec_b. No decoder matmul,
     no qT transposes.
"""
import os, sys, types

os.environ.setdefault("NEURON_RT_RESET_CORES", "1")
sys.path.insert(0, '/opt/trn_rl_repo')
import numpy as np

import concourse.bass as bass
import concourse.tile as tile
from concourse import bacc, mybir
from concourse.bass_utils import run_bass_kernel_spmd
from concourse.masks import make_identity

f32 = mybir.dt.float32
fp8 = mybir.dt.float8e4
i32 = mybir.dt.int32
u16 = mybir.dt.uint16
ALU = mybir.AluOpType
DR = mybir.MatmulPerfMode.DoubleRow
AF = mybir.ActivationFunctionType

NCORES = 8
B, N, D = 4, 4096, 512
T = B * N                 # 16384 tokens
TL = T // NCORES          # 2048 tokens per core
K = 16384                 # codebook size
NT = TL // 128            # 16 token tiles per core
NJ = D // 128             # 4 contraction tiles
SC = 1024                 # superchunk (2 psum banks)
NSC = K // SC             # 16 superchunks
NR8 = 6                   # fp8 stream rows (4 cb + 2 bias-partial)
NUM_Q = 2
NCAND = 6                 # rescued candidates per token
PADW = 520                # cbx row: 512 cb + 1 bias + pad
LAM = 0.5                 # score scale so |lam*bias| fits fp8 range
EPS_SLOT = 1e-4           # slot perturbation making m8 values distinct


def _ensure_axon_hook():
    """Register the NTFF profile hook (missing antenv.axon_hooks shim)."""
    if "antenv.axon_hooks" in sys.modules:
        return
    mod = types.ModuleType("antenv.axon_hooks")
    _h = [None]
    mod.set_axon_ntff_profile_hook = lambda h: _h.__setitem__(0, h)
    mod.get_axon_ntff_profile_hook = lambda: _h[0]
    sys.modules["antenv.axon_hooks"] = mod
    try:
        import antenv
        antenv.axon_hooks = mod
        from trn_agent_boot.trn_boot import _ntff_profile_via_ctypes
        hook = _ntff_profile_via_ctypes('/opt/axon/libaxon_pjrt.so')
        if hook is not None:
            mod.set_axon_ntff_profile_hook(hook)
    except Exception:
        pass


def _build():
    nc = bacc.Bacc("TRN2", target_bir_lowering=False, debug=False,
                   num_devices=NCORES)

    xT_d = nc.dram_tensor("xT", [128, NJ, TL], f32, kind="ExternalInput")
    xT8_d = nc.dram_tensor("xT8", [128, NJ, TL], fp8, kind="ExternalInput")
    cbsE_d = nc.dram_tensor("cbsE", [128, NR8, K], fp8, kind="ExternalInput")
    cbs2_d = nc.dram_tensor("cbs2", [128, NR8, K], fp8, kind="ExternalInput")
    cbx_d = nc.dram_tensor("cbx", [K, PADW], f32, kind="ExternalInput")
    cbq_d = nc.dram_tensor("cbq", [K, D], f32, kind="ExternalInput")
    cbD_d = nc.dram_tensor("cbD", [K, D], f32, kind="ExternalInput")
    ewT_d = nc.dram_tensor("ewT", [128, NJ, D], f32, kind="ExternalInput")
    ebf_d = nc.dram_tensor("ebf", [128, D], f32, kind="ExternalInput")
    db_d = nc.dram_tensor("db", [128, D], f32, kind="ExternalInput")
    ones8_d = nc.dram_tensor("ones8", [128, 2, 128], fp8, kind="ExternalInput")
    out_d = nc.dram_tensor("out", [TL, D], f32, kind="ExternalOutput")

    from contextlib import ExitStack
    with tile.TileContext(nc) as tc, ExitStack() as ctx:
        big = ctx.enter_context(tc.tile_pool(name="big", bufs=1))
        xp = ctx.enter_context(tc.tile_pool(name="xp", bufs=1))
        cbp = ctx.enter_context(tc.tile_pool(name="cbp", bufs=3))
        q8p = ctx.enter_context(tc.tile_pool(name="q8p", bufs=3))
        qtp = ctx.enter_context(tc.tile_pool(name="qtp", bufs=3))
        drp = ctx.enter_context(tc.tile_pool(name="drp", bufs=3))
        outp = ctx.enter_context(tc.tile_pool(name="outp", bufs=2))
        scrp = ctx.enter_context(tc.tile_pool(name="scr", bufs=2))
        smallp = ctx.enter_context(tc.tile_pool(name="small", bufs=8))
        psc = ctx.enter_context(tc.tile_pool(name="psc", bufs=3, space="PSUM"))
        psm = ctx.enter_context(tc.tile_pool(name="psm", bufs=2, space="PSUM"))

        # ---- persistent tiles
        xT8 = big.tile([128, NJ, TL], fp8)     # stage-1 lhsT (fp8 of x)
        rT8 = big.tile([128, NJ, TL], fp8)     # stage-2 lhsT (fp8 residual)
        z_td = big.tile([128, NT, D], f32)     # exact residual, [t, d] layout
        qacc = big.tile([128, NT, D], f32)     # output accumulator
        ewT = big.tile([128, NJ, D], f32)
        ebf = big.tile([128, D], f32)
        db = big.tile([128, D], f32)
        ones8 = big.tile([128, 2, 128], fp8)
        ident = big.tile([128, 128], f32)
        m8buf = big.tile([128, NT, NSC * 8], f32)
        idxbuf = big.tile([128, NT, NSC * 8], u16)
        iota_off8 = big.tile([128, NSC * 8], f32)
        iota_eps = big.tile([128, NSC * 8], f32)

        nc.sync.dma_start(xT8[:], xT8_d.ap())
        nc.sync.dma_start(ewT[:], ewT_d.ap())
        nc.sync.dma_start(ebf[:], ebf_d.ap())
        nc.sync.dma_start(db[:], db_d.ap())
        nc.sync.dma_start(ones8[:], ones8_d.ap())
        make_identity(nc, ident[:])
        nc.gpsimd.iota(iota_off8[:], pattern=[[SC, NSC], [0, 8]], base=0,
                       channel_multiplier=0,
                       allow_small_or_imprecise_dtypes=True)
        nc.gpsimd.iota(iota_eps[:], pattern=[[1, NSC * 8]], base=0,
                       channel_multiplier=0,
                       allow_small_or_imprecise_dtypes=True)
        nc.scalar.mul(iota_eps[:], iota_eps[:], EPS_SLOT)

        # ---- encoder (overlapped under stage-1 sweep): z_td = x @ ewT + eb
        HL = TL // 2
        xth = [None, None]

        def emit_enc_dma(half):
            xth[half] = xp.tile([128, NJ, HL], f32, tag="x", name="xth")
            nc.sync.dma_start(xth[half][:],
                              xT_d.ap()[:, :, half * HL:(half + 1) * HL])

        def emit_enc_tile(t):
            half, t2 = divmod(t, HL // 128)
            psz = psm.tile([128, 512], f32, tag="m")
            for j in range(NJ):
                nc.tensor.matmul(
                    psz[:], lhsT=xth[half][:, j, t2 * 128:(t2 + 1) * 128],
                    rhs=ewT[:, j, :],
                    start=(j == 0), stop=(j == NJ - 1))
            nc.vector.tensor_add(z_td[:, t, :], psz[:], ebf[:])

        def emit_rt8(t):
            """rT8[:, :, t*128:(t+1)*128] = fp8(transpose(z_td[:, t, :]))."""
            tp = psm.tile([128, NJ, 128], f32, tag="m")
            for j in range(NJ):
                nc.tensor.transpose(tp[:, j, :],
                                    z_td[:, t, j * 128:(j + 1) * 128],
                                    ident[:])
            nc.scalar.copy(rT8[:, :, t * 128:(t + 1) * 128], tp[:])

        # ---- fp8 sweep + scans
        lhs = [xT8, rT8]
        cbs_d = [cbsE_d, cbs2_d]

        def emit_sweep_sc(s, tset, sc):
            cbt = cbp.tile([128, NR8, SC], fp8, tag="cbt")
            nc.sync.dma_start(cbt[:], cbs_d[s].ap()[:, :, sc * SC:(sc + 1) * SC])
            lh = lhs[s]
            for t in tset:
                tsl = slice(t * 128, (t + 1) * 128)
                ps = psc.tile([128, SC], f32, tag="sc")
                for h in range(SC // 512):
                    pslice = ps[:, h * 512:(h + 1) * 512]
                    cslice = slice(h * 512, (h + 1) * 512)
                    nc.tensor.matmul(
                        pslice, lhsT=lh[:, 0:2, tsl],
                        rhs=cbt[:, 0:2, cslice],
                        start=True, stop=False, perf_mode=DR)
                    nc.tensor.matmul(
                        pslice, lhsT=lh[:, 2:4, tsl],
                        rhs=cbt[:, 2:4, cslice],
                        start=False, stop=False, perf_mode=DR)
                    nc.tensor.matmul(
                        pslice, lhsT=ones8[:],
                        rhs=cbt[:, 4:6, cslice],
                        start=False, stop=True, perf_mode=DR)
                m8s = m8buf[:, t, sc * 8:sc * 8 + 8]
                nc.vector.max(out=m8s, in_=ps[:])
                nc.vector.max_index(out=idxbuf[:, t, sc * 8:sc * 8 + 8],
                                    in_max=m8s, in_values=ps[:])

        # ---- rescue pipeline (per token tile, per stage)
        comb = {}

        def emit_a(s, t):
            st = {}
            # distinct-valued selection array
            m8p = scrp.tile([128, NSC * 8], f32, tag="m8p")
            nc.vector.scalar_tensor_tensor(
                out=m8p[:], in0=m8buf[:, t, :], scalar=1.0,
                in1=iota_eps[:], op0=ALU.mult, op1=ALU.add)
            g8 = smallp.tile([128, 8], f32, tag="g8")
            nc.vector.max(out=g8[:], in_=m8p[:])
            # absolute code ids
            idxf = smallp.tile([128, NSC * 8], f32, tag="idxf")
            nc.gpsimd.tensor_copy(idxf[:], idxbuf[:, t, :])
            nc.gpsimd.tensor_add(idxf[:], idxf[:], iota_off8[:])
            # candidate ids via cumulative is_ge matching (m8p values distinct)
            cs = smallp.tile([128, NCAND], f32, tag="cs")
            junkm = scrp.tile([128, NSC * 8], f32, tag="jm")
            for k in range(NCAND):
                nc.vector.scalar_tensor_tensor(
                    out=junkm[:], in0=m8p[:],
                    scalar=g8[:, k:k + 1], in1=idxf[:],
                    op0=ALU.is_ge, op1=ALU.mult,
                    accum_out=cs[:, k:k + 1])
            candf = smallp.tile([128, NCAND], f32, tag="cf")
            nc.vector.tensor_copy(candf[:, 0:1], cs[:, 0:1])
            nc.vector.tensor_sub(candf[:, 1:NCAND], cs[:, 1:NCAND],
                                 cs[:, 0:NCAND - 1])
            cand32 = smallp.tile([128, NCAND], i32, tag="c32")
            nc.vector.tensor_copy(cand32[:], candf[:])
            q8 = q8p.tile([128, NCAND, PADW], f32, tag="q8")
            for k in range(NCAND):
                nc.gpsimd.indirect_dma_start(
                    out=q8[:, k, :], out_offset=None, in_=cbx_d.ap(),
                    in_offset=bass.IndirectOffsetOnAxis(
                        ap=cand32[:, k:k + 1], axis=0))
            st["candf"], st["q8"] = candf, q8
            comb[(s, t)] = st

        def emit_b(s, t):
            st = comb[(s, t)]
            candf, q8 = st["candf"], st["q8"]
            sc8 = smallp.tile([128, NCAND], f32, tag="sc8")
            for k in range(NCAND):
                junk2 = scrp.tile([128, D], f32, tag="j2")
                nc.gpsimd.tensor_mul(junk2[:], q8[:, k, 0:D],
                                     z_td[:, t, :])
                nc.scalar.activation(
                    out=junk2[:], in_=junk2[:], func=AF.Copy,
                    accum_out=sc8[:, k:k + 1])
            nc.vector.tensor_add(sc8[:], sc8[:],
                                 q8[:, :, D:D + 1].squeeze())
            gm = smallp.tile([128, 1], f32, tag="gm")
            nc.vector.tensor_reduce(gm[:], sc8[:],
                                    axis=mybir.AxisListType.X,
                                    op=ALU.max)
            junk8 = smallp.tile([128, NCAND], f32, tag="j8")
            wf = smallp.tile([128, 1], f32, tag="wf")
            nc.vector.scalar_tensor_tensor(
                out=junk8[:], in0=sc8[:], scalar=gm[:],
                in1=candf[:], op0=ALU.is_ge, op1=ALU.mult,
                accum_out=wf[:])
            wi32 = smallp.tile([128, 1], i32, tag="wi")
            nc.vector.tensor_copy(wi32[:], wf[:])
            drow = drp.tile([128, D], f32, tag="dr")
            nc.gpsimd.indirect_dma_start(
                out=drow[:], out_offset=None, in_=cbD_d.ap(),
                in_offset=bass.IndirectOffsetOnAxis(ap=wi32[:], axis=0))
            st["drow"] = drow
            if s == 0:
                qrow = qtp.tile([128, D], f32, tag="qt")
                nc.gpsimd.indirect_dma_start(
                    out=qrow[:], out_offset=None,
                    in_=cbq_d.ap(),
                    in_offset=bass.IndirectOffsetOnAxis(ap=wi32[:], axis=0))
                st["qrow"] = qrow

        def emit_c(s, t):
            st = comb.pop((s, t))
            drow = st["drow"]
            if s == 0:
                nc.gpsimd.tensor_sub(z_td[:, t, :], z_td[:, t, :],
                                     st["qrow"][:])
                emit_rt8(t)
                nc.gpsimd.tensor_add(qacc[:, t, :], drow[:], db[:])
            else:
                o_t = outp.tile([128, D], f32, tag="o")
                nc.gpsimd.tensor_add(o_t[:], qacc[:, t, :], drow[:])
                nc.sync.dma_start(out_d.ap()[t * 128:(t + 1) * 128, :], o_t[:])

        def pump(ps, tiles, r, coff=5):
            n = len(tiles)
            if 0 <= r < n:
                emit_a(ps, tiles[r])
            if 0 <= r - 2 < n:
                emit_b(ps, tiles[r - 2])
            if 0 <= r - coff < n:
                emit_c(ps, tiles[r - coff])

        GROUPS = [range(0, 8), range(8, 16)]
        emit_enc_dma(0)
        prev = None
        w = 0
        for s in range(NUM_Q):
            for tset in GROUPS:
                for sc in range(NSC):
                    emit_sweep_sc(s, tset, sc)
                    if prev is not None:
                        pump(prev[0], prev[1], sc)
                    if s == 0:
                        if w == 18:
                            emit_enc_dma(1)
                        if 3 <= w <= 17 and w % 2 == 1:
                            emit_enc_tile((w - 3) // 2)
                        elif 21 <= w <= 28:
                            emit_enc_tile(w - 13)
                        w += 1
                prev = (s, list(tset))

        # tail flush: tight pump cadence, gathers pipelined at depth 2
        ps_, tiles_ = prev
        for r in range(len(tiles_) + 5):
            pump(ps_, tiles_, r, coff=4)

    nc.compile()
    return nc


_CACHE = {}


def _get_nc():
    if "nc" not in _CACHE:
        _ensure_axon_hook()
        _CACHE["nc"] = _build()
    return _CACHE["nc"]


def _host_prep(x, enc_w, enc_b, codebook, dec_w, dec_b):
    import ml_dtypes
    f8 = ml_dtypes.float8_e4m3

    x = np.asarray(x, np.float32)
    enc_w = np.asarray(enc_w, np.float32)
    enc_b = np.asarray(enc_b, np.float32)
    cb = np.ascontiguousarray(np.asarray(codebook, np.float32))
    dec_w = np.asarray(dec_w, np.float32)
    dec_b = np.asarray(dec_b, np.float32)

    flat = x.reshape(T, D)
    csq = (cb.astype(np.float64) ** 2).sum(-1).astype(np.float32)
    bias = (-0.5 * csq).astype(np.float32)

    def fp8_stream(table, bias_vec):
        """[128, NR8, K] fp8: rows 0-3 = lam*table.T k-tiles, 4-5 = bias."""
        tT = np.ascontiguousarray(table.T)                # [D, K]
        cbs = np.zeros((128, NR8, K), np.float32)
        cbs[:, :NJ, :] = (LAM * tT).reshape(NJ, 128, K).transpose(1, 0, 2)
        rem = (LAM * bias_vec).astype(np.float32).copy()
        parts = []
        for _ in range(4):
            p = np.asarray(rem, f8).astype(np.float32)
            parts.append(p)
            rem = rem - p
        cbs[0, NJ, :] = parts[0]
        cbs[1, NJ, :] = parts[1]
        cbs[0, NJ + 1, :] = parts[2]
        cbs[1, NJ + 1, :] = parts[3]
        return np.asarray(cbs, f8)

    # stage-1 sweeps on x directly: z.c == x.(cb @ enc_w) + enc_b.c
    cbE = (cb @ enc_w).astype(np.float32)
    biasE = bias + (cb @ enc_b).astype(np.float32)
    cbsE = fp8_stream(cbE, biasE)
    cbs2 = fp8_stream(cb, bias)

    # rescore table [cb row | bias | pad]; decoder-folded gather table
    cbx = np.zeros((K, PADW), np.float32)
    cbx[:, :D] = cb
    cbx[:, D] = bias
    cbD = np.ascontiguousarray(cb @ dec_w.T.astype(np.float32))
    cbq = cb

    ewT = np.ascontiguousarray(
        enc_w.T.reshape(NJ, 128, D).transpose(1, 0, 2))   # [128, NJ, D]
    ebf = np.ascontiguousarray(np.broadcast_to(enc_b, (128, D)))
    dbf = np.ascontiguousarray(np.broadcast_to(dec_b, (128, D)))
    ones8 = np.ones((128, 2, 128), f8)

    common = {"cbsE": cbsE, "cbs2": cbs2, "cbx": cbx, "cbq": cbq,
              "cbD": cbD, "ewT": ewT, "ebf": ebf, "db": dbf, "ones8": ones8}

    in_maps = []
    for sh in range(NCORES):
        shard = flat[sh * TL:(sh + 1) * TL]               # [TL, D]
        xT = np.ascontiguousarray(
            shard.T.reshape(NJ, 128, TL).transpose(1, 0, 2))
        xT8 = np.asarray(xT, f8)
        in_maps.append({"xT": xT, "xT8": xT8, **common})
    return in_maps


def _run(inputs, trace=False):
    nc = _get_nc()
    in_maps = _host_prep(**inputs)
    res = run_bass_kernel_spmd(nc, in_maps, list(range(NCORES)), trace=trace)
    outs = [res.results[s]["out"] for s in range(NCORES)]
    full = np.concatenate(outs, axis=0).reshape(B, N, D)
    return full, res


def kernel(**inputs) -> np.ndarray:
    out, _ = _run(inputs, trace=False)
    return out


def kernel_traced(**inputs):
    out, res = _run(inputs, trace=True)
    return out, res


# revision 21
# speedup vs baseline: 1.1282x; 1.0837x over previous
"""Trainium2 Bass kernel for residual-VQ autoencoder (nn_Autoencoder_45148696216751).

v3: encoder/decoder folding + DVE-minimal rescue.

Per core (data-parallel over tokens, 8 cores x 2048 tokens):
  stage-1 sweep runs directly on fp8(x) against cbE = codebook @ enc_w
  (host-folded), so the exact f32 encoder (z = x @ enc_w.T + b) overlaps
  under the sweep instead of gating it.
  2x VQ stage:
     scores[t,k] = lam*(r.c - |c|^2/2) via fp8 DoubleRow matmuls
     (4 data + 2 bias-partial DR matmuls per 1024-code superchunk)
     DVE: max8 -> m8buf (top-8 per superchunk, exact f32) and
          find_index8 -> idxbuf (within-superchunk positions)
     rescue per token tile: eps-perturbed m8 -> global max8 + find_index8
     gives the top-8 SLOTS; gpsimd gathers absolute code ids by slot;
     ONE batched indirect-DMA gather of 6 candidate rows [cb|bias];
     exact f32 rescore dots on gpsimd/scalar; winner picked by is_ge
     match; winner row gathered from cbx (residual update) and from
     cbD = codebook @ dec_w.T (host-folded decoder) for the output
     accumulation out[t] = cbD[w1] + cbD[w2] + dec_b. No decoder matmul,
     no qT transposes.
"""
import os, sys, types

os.environ.setdefault("NEURON_RT_RESET_CORES", "1")
sys.path.insert(0, '/opt/trn_rl_repo')
import numpy as np

import concourse.bass as bass
import concourse.tile as tile
from concourse import bacc, mybir
from concourse.bass_utils import run_bass_kernel_spmd
from concourse.masks import make_identity

f32 = mybir.dt.float32
fp8 = mybir.dt.float8e4
i32 = mybir.dt.int32
u16 = mybir.dt.uint16
ALU = mybir.AluOpType
DR = mybir.MatmulPerfMode.DoubleRow
AF = mybir.ActivationFunctionType

NCORES = 8
B, N, D = 4, 4096, 512
T = B * N                 # 16384 tokens
TL = T // NCORES          # 2048 tokens per core
K = 16384                 # codebook size
NT = TL // 128            # 16 token tiles per core
NJ = D // 128             # 4 contraction tiles
SC = 1024                 # superchunk (2 psum banks)
NSC = K // SC             # 16 superchunks
NR8 = 6                   # fp8 stream rows (4 cb + 2 bias-partial)
NUM_Q = 2
NCAND = 5                 # rescued candidates per token
PADW = 520                # cbx row: 512 cb + 1 bias + pad
LAM = 0.5                 # score scale so |lam*bias| fits fp8 range
EPS_SLOT = 1e-4           # slot perturbation making m8 values distinct


def _ensure_axon_hook():
    """Register the NTFF profile hook (missing antenv.axon_hooks shim)."""
    if "antenv.axon_hooks" in sys.modules:
        return
    mod = types.ModuleType("antenv.axon_hooks")
    _h = [None]
    mod.set_axon_ntff_profile_hook = lambda h: _h.__setitem__(0, h)
    mod.get_axon_ntff_profile_hook = lambda: _h[0]
    sys.modules["antenv.axon_hooks"] = mod
    try:
        import antenv
        antenv.axon_hooks = mod
        from trn_agent_boot.trn_boot import _ntff_profile_via_ctypes
        hook = _ntff_profile_via_ctypes('/opt/axon/libaxon_pjrt.so')
        if hook is not None:
            mod.set_axon_ntff_profile_hook(hook)
    except Exception:
        pass


def _build():
    nc = bacc.Bacc("TRN2", target_bir_lowering=False, debug=False,
                   num_devices=NCORES)

    xT_d = nc.dram_tensor("xT", [128, NJ, TL], f32, kind="ExternalInput")
    xT8_d = nc.dram_tensor("xT8", [128, NJ, TL], fp8, kind="ExternalInput")
    cbsE_d = nc.dram_tensor("cbsE", [128, NR8, K], fp8, kind="ExternalInput")
    cbs2_d = nc.dram_tensor("cbs2", [128, NR8, K], fp8, kind="ExternalInput")
    cbx_d = nc.dram_tensor("cbx", [K, PADW], f32, kind="ExternalInput")
    cbq_d = nc.dram_tensor("cbq", [K, D], f32, kind="ExternalInput")
    cbD_d = nc.dram_tensor("cbD", [K, D], f32, kind="ExternalInput")
    ewT_d = nc.dram_tensor("ewT", [128, NJ, D], f32, kind="ExternalInput")
    ebf_d = nc.dram_tensor("ebf", [128, D], f32, kind="ExternalInput")
    db_d = nc.dram_tensor("db", [128, D], f32, kind="ExternalInput")
    ones8_d = nc.dram_tensor("ones8", [128, 2, 128], fp8, kind="ExternalInput")
    out_d = nc.dram_tensor("out", [TL, D], f32, kind="ExternalOutput")

    from contextlib import ExitStack
    with tile.TileContext(nc) as tc, ExitStack() as ctx:
        big = ctx.enter_context(tc.tile_pool(name="big", bufs=1))
        xp = ctx.enter_context(tc.tile_pool(name="xp", bufs=1))
        cbp = ctx.enter_context(tc.tile_pool(name="cbp", bufs=3))
        q8p = ctx.enter_context(tc.tile_pool(name="q8p", bufs=3))
        qtp = ctx.enter_context(tc.tile_pool(name="qtp", bufs=3))
        drp = ctx.enter_context(tc.tile_pool(name="drp", bufs=3))
        outp = ctx.enter_context(tc.tile_pool(name="outp", bufs=2))
        scrp = ctx.enter_context(tc.tile_pool(name="scr", bufs=2))
        smallp = ctx.enter_context(tc.tile_pool(name="small", bufs=8))
        psc = ctx.enter_context(tc.tile_pool(name="psc", bufs=3, space="PSUM"))
        psm = ctx.enter_context(tc.tile_pool(name="psm", bufs=2, space="PSUM"))

        # ---- persistent tiles
        xT8 = big.tile([128, NJ, TL], fp8)     # stage-1 lhsT (fp8 of x)
        rT8 = big.tile([128, NJ, TL], fp8)     # stage-2 lhsT (fp8 residual)
        z_td = big.tile([128, NT, D], f32)     # exact residual, [t, d] layout
        qacc = big.tile([128, NT, D], f32)     # output accumulator
        ewT = big.tile([128, NJ, D], f32)
        ebf = big.tile([128, D], f32)
        db = big.tile([128, D], f32)
        ones8 = big.tile([128, 2, 128], fp8)
        ident = big.tile([128, 128], f32)
        m8buf = big.tile([128, NT, NSC * 8], f32)
        idxbuf = big.tile([128, NT, NSC * 8], u16)
        iota_off8 = big.tile([128, NSC * 8], f32)

        nc.sync.dma_start(xT8[:], xT8_d.ap())
        nc.sync.dma_start(ewT[:], ewT_d.ap())
        nc.sync.dma_start(ebf[:], ebf_d.ap())
        nc.sync.dma_start(db[:], db_d.ap())
        nc.sync.dma_start(ones8[:], ones8_d.ap())
        make_identity(nc, ident[:])
        nc.gpsimd.iota(iota_off8[:], pattern=[[SC, NSC], [0, 8]], base=0,
                       channel_multiplier=0,
                       allow_small_or_imprecise_dtypes=True)


        # ---- encoder (overlapped under stage-1 sweep): z_td = x @ ewT + eb
        HL = TL // 2
        xth = [None, None]

        def emit_enc_dma(half):
            xth[half] = xp.tile([128, NJ, HL], f32, tag="x", name="xth")
            nc.sync.dma_start(xth[half][:],
                              xT_d.ap()[:, :, half * HL:(half + 1) * HL])

        def emit_enc_tile(t):
            half, t2 = divmod(t, HL // 128)
            psz = psm.tile([128, 512], f32, tag="m")
            for j in range(NJ):
                nc.tensor.matmul(
                    psz[:], lhsT=xth[half][:, j, t2 * 128:(t2 + 1) * 128],
                    rhs=ewT[:, j, :],
                    start=(j == 0), stop=(j == NJ - 1))
            nc.scalar.copy(z_td[:, t, :], psz[:])
            nc.gpsimd.tensor_add(z_td[:, t, :], z_td[:, t, :], ebf[:])

        def emit_rt8(t):
            """rT8[:, :, t*128:(t+1)*128] = fp8(transpose(z_td[:, t, :]))."""
            tp = psm.tile([128, NJ, 128], f32, tag="m")
            for j in range(NJ):
                nc.tensor.transpose(tp[:, j, :],
                                    z_td[:, t, j * 128:(j + 1) * 128],
                                    ident[:])
            nc.scalar.copy(rT8[:, :, t * 128:(t + 1) * 128], tp[:])

        # ---- fp8 sweep + scans
        lhs = [xT8, rT8]
        cbs_d = [cbsE_d, cbs2_d]

        def emit_sweep_sc(s, tset, sc):
            cbt = cbp.tile([128, NR8, SC], fp8, tag="cbt")
            nc.sync.dma_start(cbt[:], cbs_d[s].ap()[:, :, sc * SC:(sc + 1) * SC])
            lh = lhs[s]
            for t in tset:
                tsl = slice(t * 128, (t + 1) * 128)
                ps = psc.tile([128, SC], f32, tag="sc")
                for h in range(SC // 512):
                    pslice = ps[:, h * 512:(h + 1) * 512]
                    cslice = slice(h * 512, (h + 1) * 512)
                    nc.tensor.matmul(
                        pslice, lhsT=lh[:, 0:2, tsl],
                        rhs=cbt[:, 0:2, cslice],
                        start=True, stop=False, perf_mode=DR)
                    nc.tensor.matmul(
                        pslice, lhsT=lh[:, 2:4, tsl],
                        rhs=cbt[:, 2:4, cslice],
                        start=False, stop=False, perf_mode=DR)
                    nc.tensor.matmul(
                        pslice, lhsT=ones8[:],
                        rhs=cbt[:, 4:6, cslice],
                        start=False, stop=True, perf_mode=DR)
                m8s = m8buf[:, t, sc * 8:sc * 8 + 8]
                nc.vector.max(out=m8s, in_=ps[:])
                nc.vector.max_index(out=idxbuf[:, t, sc * 8:sc * 8 + 8],
                                    in_max=m8s, in_values=ps[:])

        # ---- rescue pipeline (per token tile, per stage)
        comb = {}

        def emit_a(s, t):
            st = {}
            g8 = smallp.tile([128, 8], f32, tag="g8")
            nc.vector.max(out=g8[:], in_=m8buf[:, t, :])
            # absolute code ids
            idxf = smallp.tile([128, NSC * 8], f32, tag="idxf")
            nc.gpsimd.tensor_copy(idxf[:], idxbuf[:, t, :])
            nc.gpsimd.tensor_add(idxf[:], idxf[:], iota_off8[:])
            # candidate ids via cumulative is_ge matching
            cs = smallp.tile([128, NCAND], f32, tag="cs")
            junkm = scrp.tile([128, NSC * 8], f32, tag="jm")
            for k in range(NCAND):
                nc.vector.scalar_tensor_tensor(
                    out=junkm[:], in0=m8buf[:, t, :],
                    scalar=g8[:, k:k + 1], in1=idxf[:],
                    op0=ALU.is_ge, op1=ALU.mult,
                    accum_out=cs[:, k:k + 1])
            candf = smallp.tile([128, NCAND], f32, tag="cf")
            nc.vector.tensor_copy(candf[:, 0:1], cs[:, 0:1])
            nc.vector.tensor_sub(candf[:, 1:NCAND], cs[:, 1:NCAND],
                                 cs[:, 0:NCAND - 1])
            cand32 = smallp.tile([128, NCAND], i32, tag="c32")
            nc.vector.tensor_copy(cand32[:], candf[:])
            q8 = q8p.tile([128, NCAND, PADW], f32, tag="q8")
            for k in range(NCAND):
                nc.gpsimd.indirect_dma_start(
                    out=q8[:, k, :], out_offset=None, in_=cbx_d.ap(),
                    in_offset=bass.IndirectOffsetOnAxis(
                        ap=cand32[:, k:k + 1], axis=0))
            st["candf"], st["q8"] = candf, q8
            comb[(s, t)] = st

        def emit_b(s, t):
            st = comb[(s, t)]
            candf, q8 = st["candf"], st["q8"]
            sc8 = smallp.tile([128, NCAND], f32, tag="sc8")
            for k in range(NCAND):
                junk2 = scrp.tile([128, D], f32, tag="j2")
                nc.gpsimd.tensor_mul(junk2[:], q8[:, k, 0:D],
                                     z_td[:, t, :])
                nc.scalar.activation(
                    out=junk2[:], in_=junk2[:], func=AF.Copy,
                    accum_out=sc8[:, k:k + 1])
            nc.vector.tensor_add(sc8[:], sc8[:],
                                 q8[:, :, D:D + 1].squeeze())
            gm = smallp.tile([128, 1], f32, tag="gm")
            nc.vector.tensor_reduce(gm[:], sc8[:],
                                    axis=mybir.AxisListType.X,
                                    op=ALU.max)
            junk8 = smallp.tile([128, NCAND], f32, tag="j8")
            wf = smallp.tile([128, 1], f32, tag="wf")
            nc.vector.scalar_tensor_tensor(
                out=junk8[:], in0=sc8[:], scalar=gm[:],
                in1=candf[:], op0=ALU.is_ge, op1=ALU.mult,
                accum_out=wf[:])
            wi32 = smallp.tile([128, 1], i32, tag="wi")
            nc.vector.tensor_copy(wi32[:], wf[:])
            drow = drp.tile([128, D], f32, tag="dr")
            nc.gpsimd.indirect_dma_start(
                out=drow[:], out_offset=None, in_=cbD_d.ap(),
                in_offset=bass.IndirectOffsetOnAxis(ap=wi32[:], axis=0))
            st["drow"] = drow
            if s == 0:
                qrow = qtp.tile([128, D], f32, tag="qt")
                nc.gpsimd.indirect_dma_start(
                    out=qrow[:], out_offset=None,
                    in_=cbq_d.ap(),
                    in_offset=bass.IndirectOffsetOnAxis(ap=wi32[:], axis=0))
                st["qrow"] = qrow

        def emit_c(s, t):
            st = comb.pop((s, t))
            drow = st["drow"]
            if s == 0:
                nc.gpsimd.tensor_sub(z_td[:, t, :], z_td[:, t, :],
                                     st["qrow"][:])
                emit_rt8(t)
                nc.gpsimd.tensor_add(qacc[:, t, :], drow[:], db[:])
            else:
                o_t = outp.tile([128, D], f32, tag="o")
                nc.gpsimd.tensor_add(o_t[:], qacc[:, t, :], drow[:])
                nc.sync.dma_start(out_d.ap()[t * 128:(t + 1) * 128, :], o_t[:])

        def pump(ps, tiles, r, coff=5):
            n = len(tiles)
            if 0 <= r < n:
                emit_a(ps, tiles[r])
            if 0 <= r - 2 < n:
                emit_b(ps, tiles[r - 2])
            if 0 <= r - coff < n:
                emit_c(ps, tiles[r - coff])

        GROUPS = [range(0, 8), range(8, 16)]
        emit_enc_dma(0)
        prev = None
        w = 0
        for s in range(NUM_Q):
            for tset in GROUPS:
                for sc in range(NSC):
                    emit_sweep_sc(s, tset, sc)
                    if prev is not None:
                        pump(prev[0], prev[1], sc)
                    if s == 0:
                        if w == 18:
                            emit_enc_dma(1)
                        if 3 <= w <= 17 and w % 2 == 1:
                            emit_enc_tile((w - 3) // 2)
                        elif 21 <= w <= 28:
                            emit_enc_tile(w - 13)
                        w += 1
                prev = (s, list(tset))

        # tail flush: tight pump cadence, gathers pipelined at depth 2
        ps_, tiles_ = prev
        for r in range(len(tiles_) + 5):
            pump(ps_, tiles_, r, coff=4)

    nc.compile()
    return nc


_CACHE = {}


def _get_nc():
    if "nc" not in _CACHE:
        _ensure_axon_hook()
        _CACHE["nc"] = _build()
    return _CACHE["nc"]


def _host_prep(x, enc_w, enc_b, codebook, dec_w, dec_b):
    import ml_dtypes
    f8 = ml_dtypes.float8_e4m3

    x = np.asarray(x, np.float32)
    enc_w = np.asarray(enc_w, np.float32)
    enc_b = np.asarray(enc_b, np.float32)
    cb = np.ascontiguousarray(np.asarray(codebook, np.float32))
    dec_w = np.asarray(dec_w, np.float32)
    dec_b = np.asarray(dec_b, np.float32)

    flat = x.reshape(T, D)
    csq = (cb.astype(np.float64) ** 2).sum(-1).astype(np.float32)
    bias = (-0.5 * csq).astype(np.float32)

    def fp8_stream(table, bias_vec):
        """[128, NR8, K] fp8: rows 0-3 = lam*table.T k-tiles, 4-5 = bias."""
        tT = np.ascontiguousarray(table.T)                # [D, K]
        cbs = np.zeros((128, NR8, K), np.float32)
        cbs[:, :NJ, :] = (LAM * tT).reshape(NJ, 128, K).transpose(1, 0, 2)
        rem = (LAM * bias_vec).astype(np.float32).copy()
        parts = []
        for _ in range(4):
            p = np.asarray(rem, f8).astype(np.float32)
            parts.append(p)
            rem = rem - p
        cbs[0, NJ, :] = parts[0]
        cbs[1, NJ, :] = parts[1]
        cbs[0, NJ + 1, :] = parts[2]
        cbs[1, NJ + 1, :] = parts[3]
        return np.asarray(cbs, f8)

    # stage-1 sweeps on x directly: z.c == x.(cb @ enc_w) + enc_b.c
    cbE = (cb @ enc_w).astype(np.float32)
    biasE = bias + (cb @ enc_b).astype(np.float32)
    cbsE = fp8_stream(cbE, biasE)
    cbs2 = fp8_stream(cb, bias)

    # rescore table [cb row | bias | pad]; decoder-folded gather table
    cbx = np.zeros((K, PADW), np.float32)
    cbx[:, :D] = cb
    cbx[:, D] = bias
    cbD = np.ascontiguousarray(cb @ dec_w.T.astype(np.float32))
    cbq = cb

    ewT = np.ascontiguousarray(
        enc_w.T.reshape(NJ, 128, D).transpose(1, 0, 2))   # [128, NJ, D]
    ebf = np.ascontiguousarray(np.broadcast_to(enc_b, (128, D)))
    dbf = np.ascontiguousarray(np.broadcast_to(dec_b, (128, D)))
    ones8 = np.ones((128, 2, 128), f8)

    common = {"cbsE": cbsE, "cbs2": cbs2, "cbx": cbx, "cbq": cbq,
              "cbD": cbD, "ewT": ewT, "ebf": ebf, "db": dbf, "ones8": ones8}

    in_maps = []
    for sh in range(NCORES):
        shard = flat[sh * TL:(sh + 1) * TL]               # [TL, D]
        xT = np.ascontiguousarray(
            shard.T.reshape(NJ, 128, TL).transpose(1, 0, 2))
        xT8 = np.asarray(xT, f8)
        in_maps.append({"xT": xT, "xT8": xT8, **common})
    return in_maps


def _run(inputs, trace=False):
    nc = _get_nc()
    in_maps = _host_prep(**inputs)
    res = run_bass_kernel_spmd(nc, in_maps, list(range(NCORES)), trace=trace)
    outs = [res.results[s]["out"] for s in range(NCORES)]
    full = np.concatenate(outs, axis=0).reshape(B, N, D)
    return full, res


def kernel(**inputs) -> np.ndarray:
    out, _ = _run(inputs, trace=False)
    return out


def kernel_traced(**inputs):
    out, res = _run(inputs, trace=True)
    return out, res
